# revision 1
# baseline (speedup 1.0000x reference)
"""Trainium2 Bass kernel for nn_DirectInjectionEncoder (moe_routing).

Strategy (8 NeuronCores):
  - The three big projection GEMMs (Wgate/Wup/Wdown, 10240->2560) are
    sharded over the output dim d_model=2560 -> 320 columns per core, so
    each core streams only 1/8 of the big weights from HBM. Every core
    computes its 320-column slice of all 16*36=576 rows per group.
  - Row L2-norms need the full 2560-dim row, so each core computes partial
    sums of squares for its slice; one tiny 8-core AllGather (15 cols x 128
    partitions) distributes the partials and every core reconstructs the
    full norm locally before scaling its slice.
  - The small projections (Wk/Wv, 640->2560) are data-parallel over the
    batch (weights replicated, norms core-local), and run together with the
    identity tokens inside the AllGather's latency window.
  - Identity tokens (9 of 14 slots/layer, first 2560 dims, no weights) are
    data-parallel over the batch: core c handles batches [2c, 2c+1] fully.
  - Host-side prep ("sharding") gathers token groups, pre-transposes the
    contraction dim onto partitions, and slices the weights per core.
  - GEMM operands stream as bf16 (KERNEL_GEMM_DT=f32r/f32 to override);
    PSUM accumulation and the norm math stay fp32. fp32 matmul runs at 1/4
    PE rate on TRN2, so bf16/f32r is 4x PE throughput; bf16 also halves
    the dominant DMA traffic.
"""

import os
import sys

sys.path.insert(0, "/opt/trn_rl_repo")

import numpy as np
import ml_dtypes

from concourse import bacc, bass, mybir
from concourse.bass_utils import run_bass_kernel_spmd
from concourse.tile import TileContext

D_MODEL = 2560
NUM_LAYERS = 36
TOKENS_PER_LAYER = 14
EPS = 1e-8
B = 16
N_CORES = 8
CORE_IDS = list(range(N_CORES))
D_SHARD = D_MODEL // N_CORES  # 320
ROWS = B * NUM_LAYERS  # 576
ROW_TILES = [(0, 128), (128, 128), (256, 128), (384, 128), (512, 64)]
ROWS_PC = ROWS // N_CORES  # 72 rows/core for the batch-parallel paths

IDENTITY_OFFSETS = np.array([0, 1, 2, 4, 6, 7, 8, 10, 13])
BIG_GROUPS = [(11, "Wup"), (9, "Wgate"), (12, "Wdown")]  # in_dim 10240, d-sharded
KV_GROUPS = [(3, "Wk"), (5, "Wv")]  # in_dim 640, batch-parallel
KV_IND = 640
BIG_IND = 10240
N_CHUNKS = D_MODEL // 512  # 5 output chunks for the kv path

ID_ROWS = (B // N_CORES) * NUM_LAYERS * len(IDENTITY_OFFSETS)  # 648
ID_TILES = [(0, 128), (128, 128), (256, 128), (384, 128), (512, 128), (640, 8)]
N_SSQ_COLS = len(BIG_GROUPS) * len(ROW_TILES)  # 15

F32 = mybir.dt.float32
AF = mybir.ActivationFunctionType

GEMM_MODE = os.environ.get("KERNEL_GEMM_DT", "bf16")
if GEMM_MODE == "bf16":
    GEMM_DT = mybir.dt.bfloat16
    GEMM_NP = ml_dtypes.bfloat16
    KB_BIG = 4  # k-tiles per DMA super-tile (~590 KB per xt transfer)
elif GEMM_MODE == "f32r":
    GEMM_DT = mybir.dt.float32r
    GEMM_NP = np.float32
    KB_BIG = 2
else:
    GEMM_DT = mybir.dt.float32
    GEMM_NP = np.float32
    KB_BIG = 2


def _positions(offset):
    return np.arange(NUM_LAYERS) * TOKENS_PER_LAYER + offset


def build_program():
    nc = bacc.Bacc("TRN2", num_devices=N_CORES)

    xt_d, wt_d, om_d = [], [], []
    for gi, (off, wname) in enumerate(BIG_GROUPS):
        xt_d.append(nc.declare_dram_parameter(f"xt_{gi}", [BIG_IND // (128 * KB_BIG), 128, KB_BIG * ROWS], GEMM_DT, isOutput=False))
        wt_d.append(nc.declare_dram_parameter(f"wt_{gi}", [BIG_IND // (128 * KB_BIG), 128, KB_BIG * D_SHARD], GEMM_DT, isOutput=False))
        om_d.append(nc.declare_dram_parameter(f"om_{gi}", [ROWS, D_SHARD], F32, isOutput=True))
    kvx_d, kvw_d, kvo_d = [], [], []
    for gi, (off, wname) in enumerate(KV_GROUPS):
        kvx_d.append(nc.declare_dram_parameter(f"kvx_{gi}", [128, 5 * ROWS_PC], GEMM_DT, isOutput=False))
        kvw_d.append(nc.declare_dram_parameter(f"kvw_{gi}", [128, 5 * D_MODEL], GEMM_DT, isOutput=False))
        kvo_d.append(nc.declare_dram_parameter(f"kvo_{gi}", [ROWS_PC, D_MODEL], F32, isOutput=True))
    idx_d = nc.declare_dram_parameter("id_x", [ID_ROWS, D_MODEL], F32, isOutput=False)
    ido_d = nc.declare_dram_parameter("out_id", [ID_ROWS, D_MODEL], F32, isOutput=True)

    with TileContext(nc) as tc:
        with (
            tc.tile_pool(name="xt", bufs=8) as xt_pool,
            tc.tile_pool(name="wt", bufs=8) as wt_pool,
            tc.tile_pool(name="sout", bufs=N_SSQ_COLS) as sout_pool,
            tc.tile_pool(name="scr", bufs=2) as scr_pool,
            tc.tile_pool(name="kvp", bufs=2) as kv_pool,
            tc.tile_pool(name="idp", bufs=3) as id_pool,
            tc.tile_pool(name="idscr", bufs=2) as idscr_pool,
            tc.tile_pool(name="small", bufs=1) as small_pool,
            tc.tile_pool(name="ps", bufs=8, space="PSUM") as psum_pool,
            tc.tile_pool(name="dram", bufs=1, space="DRAM") as dram_pool,
        ):
            ssq = small_pool.tile([128, N_SSQ_COLS], F32, tag="ssq")
            nc.vector.memset(ssq[:], 0.0)

            # Warmup AllGather: the first collective in a NEFF pays ~60us of
            # one-time setup. Fire a tiny dummy at kernel start so that cost
            # hides under the GEMM phase and the real AllGather at the tail
            # only pays the ~15us marginal cost.
            if os.environ.get("KERNEL_NO_WARM_AG", "0") != "1":
                warm_sb = small_pool.tile([1, 16], F32, tag="warmsb")
                nc.vector.memset(warm_sb[:], 0.0)
                warm_in = dram_pool.tile([16], F32, tag="warmci")
                warm_out = dram_pool.tile([N_CORES, 16], F32, tag="warmco")
                nc.gpsimd.dma_start(out=warm_in[:], in_=warm_sb[0, :])
                nc.gpsimd.collective_compute(
                    "AllGather",
                    mybir.AluOpType.bypass,
                    ins=[warm_in.opt()],
                    outs=[warm_out.opt()],
                    replica_groups=[CORE_IDS],
                )
                nc.gpsimd.dma_start(out=warm_sb[0, :], in_=warm_out[0, :])

            # Identity tile chain: emitted interleaved at group boundaries on
            # the sync ring, so the loads slot into the GEMM DMA stream and
            # the (DVE square / ACT scale) work runs while PE is busy.
            def identity_tile(t):
                t0, tw = ID_TILES[t]
                it = id_pool.tile([128, D_MODEL], F32, tag="idp", name=f"idp_{t}")
                iscr = idscr_pool.tile([128, D_MODEL], F32, tag="idscr", name=f"idscr_{t}")
                nc.scalar.dma_start(out=it[:tw, :], in_=idx_d[t0 : t0 + tw, :])
                issq = small_pool.tile([128, 1], F32, tag=f"idssq{t}", name=f"idssq_{t}")
                nc.scalar.activation(
                    iscr[:tw, :], it[:tw, :], AF.Square,
                    accum_out=issq[:tw, :],
                )
                inorm = small_pool.tile([128, 1], F32, tag=f"idnorm{t}", name=f"idnorm_{t}")
                nc.scalar.sqrt(inorm[:tw, :], issq[:tw, :])
                nc.scalar.activation(inorm[:tw, :], inorm[:tw, :], AF.Copy, bias=EPS)
                iscale = small_pool.tile([128, 1], F32, tag=f"idscale{t}", name=f"idscale_{t}")
                nc.vector.reciprocal(iscale[:tw, :], inorm[:tw, :])
                nc.scalar.activation(
                    it[:tw, :], it[:tw, :], AF.Copy,
                    scale=iscale[:tw, :],
                )
                nc.scalar.dma_start(out=ido_d[t0 : t0 + tw, :], in_=it[:tw, :])

            # k/v operand loads, emitted early at group boundaries so the kv
            # matmuls are not blocked on DMA at the tail.
            kv_tiles = {}

            def kv_load(gi):
                kvx = kv_pool.tile([128, 5, ROWS_PC], GEMM_DT, tag="kvx", name=f"kvx_{gi}")
                kvw = kv_pool.tile([128, 5, D_MODEL], GEMM_DT, tag="kvw", name=f"kvw_{gi}")
                nc.sync.dma_start(
                    out=kvx[:], in_=kvx_d[gi].rearrange("p (k c) -> p k c", k=5)
                )
                nc.sync.dma_start(
                    out=kvw[:], in_=kvw_d[gi].rearrange("p (k c) -> p k c", k=5)
                )
                kv_tiles[gi] = (kvx, kvw)

            # ---- big groups: d-sharded GEMMs, PSUM-accumulated over k ----
            souts = {}
            for gi, (off, wname) in enumerate(BIG_GROUPS):
                nk = BIG_IND // 128
                kb = KB_BIG
                nsup = nk // kb
                ps = [
                    psum_pool.tile([128, D_SHARD], F32, tag="ps", name=f"ps_{gi}_{ri}")
                    for ri in range(len(ROW_TILES))
                ]
                xt_view = xt_d[gi].rearrange("j p (kb c) -> j p kb c", kb=kb)
                wt_view = wt_d[gi].rearrange("j p (kb c) -> j p kb c", kb=kb)
                for j in range(nsup):
                    xt = xt_pool.tile([128, kb, ROWS], GEMM_DT, tag="xt", name=f"xt_{gi}_{j}")
                    wt = wt_pool.tile([128, kb, D_SHARD], GEMM_DT, tag="wt", name=f"wt_{gi}_{j}")
                    nc.sync.dma_start(out=xt[:], in_=xt_view[j])
                    nc.sync.dma_start(out=wt[:], in_=wt_view[j])
                    for k in range(kb):
                        kt = j * kb + k
                        for r, (r0, rw) in enumerate(ROW_TILES):
                            nc.tensor.matmul(
                                ps[r][:rw, :],
                                xt[:, k, r0 : r0 + rw],
                                wt[:, k, :],
                                start=(kt == 0),
                                stop=(kt == nk - 1),
                            )
                for r, (r0, rw) in enumerate(ROW_TILES):
                    col = gi * len(ROW_TILES) + r
                    so = sout_pool.tile([128, D_SHARD], F32, tag="sout", name=f"so_{gi}_{r}")
                    scr = scr_pool.tile([128, D_SHARD], F32, tag="scr", name=f"scr_{gi}_{r}")
                    nc.vector.tensor_copy(so[:rw, :], ps[r][:rw, :])
                    nc.scalar.activation(
                        scr[:rw, :], ps[r][:rw, :], AF.Square,
                        accum_out=ssq[:rw, col : col + 1],
                    )
                    souts[(gi, r)] = so
                # boundary work: slot identity/kv loads into the DMA stream
                if gi == 0:
                    kv_load(0)
                    identity_tile(0)
                    identity_tile(1)
                elif gi == 1:
                    kv_load(1)
                    identity_tile(2)
                    identity_tile(3)

            # ---- kick off the AllGather of big-group partial sums ----
            cc_in = dram_pool.tile([128, N_SSQ_COLS], F32, tag="ccin")
            cc_out = dram_pool.tile([N_CORES, 128, N_SSQ_COLS], F32, tag="ccout")
            nc.gpsimd.dma_start(out=cc_in[:], in_=ssq[:])
            nc.gpsimd.collective_compute(
                "AllGather",
                mybir.AluOpType.bypass,
                ins=[cc_in.opt()],
                outs=[cc_out.opt()],
                replica_groups=[CORE_IDS],
            )
            identity_tile(4)
            identity_tile(5)

            # ---- k/v: batch-parallel GEMMs with core-local norms; these and
            # the identity tokens fill the AllGather latency window ----
            for gi, (off, wname) in enumerate(KV_GROUPS):
                kvx, kvw = kv_tiles[gi]
                pcs = [
                    psum_pool.tile([128, 512], F32, tag="ps", name=f"pkv_{gi}_{ci}")
                    for ci in range(N_CHUNKS)
                ]
                for k in range(5):
                    for ci in range(N_CHUNKS):
                        nc.tensor.matmul(
                            pcs[ci][:ROWS_PC, :],
                            kvx[:, k, :],
                            kvw[:, k, ci * 512 : (ci + 1) * 512],
                            start=(k == 0),
                            stop=(k == 4),
                        )
                kvssq = small_pool.tile([128, N_CHUNKS], F32, tag=f"kvssq{gi}", name=f"kvssq_{gi}")
                for ci in range(N_CHUNKS):
                    kscr = scr_pool.tile([128, 512], F32, tag="scr", name=f"kscr_{gi}_{ci}")
                    nc.scalar.activation(
                        kscr[:ROWS_PC, :], pcs[ci][:ROWS_PC, :], AF.Square,
                        accum_out=kvssq[:ROWS_PC, ci : ci + 1],
                    )
                kvs = small_pool.tile([128, 1], F32, tag=f"kvs{gi}", name=f"kvs_{gi}")
                nc.vector.reduce_sum(kvs[:ROWS_PC, :], kvssq[:ROWS_PC, :], axis=mybir.AxisListType.X)
                nc.scalar.sqrt(kvs[:ROWS_PC, :], kvs[:ROWS_PC, :])
                nc.scalar.activation(kvs[:ROWS_PC, :], kvs[:ROWS_PC, :], AF.Copy, bias=EPS)
                kvsc = small_pool.tile([128, 1], F32, tag=f"kvsc{gi}", name=f"kvsc_{gi}")
                nc.vector.reciprocal(kvsc[:ROWS_PC, :], kvs[:ROWS_PC, :])
                kvo = kv_pool.tile([128, D_MODEL], F32, tag="kvo", name=f"kvo_{gi}")
                for ci in range(N_CHUNKS):
                    nc.scalar.activation(
                        kvo[:ROWS_PC, ci * 512 : (ci + 1) * 512],
                        pcs[ci][:ROWS_PC, :],
                        AF.Copy,
                        scale=kvsc[:ROWS_PC, :],
                    )
                nc.sync.dma_start(out=kvo_d[gi][:, :], in_=kvo[:ROWS_PC, :])

            # ---- AllGather readback, total norms, final scaling ----
            # Readback + output stores ride the ACT HWDGE ring so they don't
            # queue behind the kv/identity stores on the SP ring.
            ag = small_pool.tile([128, N_CORES * N_SSQ_COLS], F32, tag="ag")
            for rr in range(N_CORES):
                nc.scalar.dma_start(
                    out=ag[:, rr * N_SSQ_COLS : (rr + 1) * N_SSQ_COLS],
                    in_=cc_out[rr, :, :],
                )
            tsq = small_pool.tile([128, N_SSQ_COLS], F32, tag="tsq")
            nc.vector.tensor_add(tsq[:], ag[:, :N_SSQ_COLS], ag[:, N_SSQ_COLS : 2 * N_SSQ_COLS])
            for rr in range(2, N_CORES):
                nc.vector.tensor_add(
                    tsq[:], tsq[:], ag[:, rr * N_SSQ_COLS : (rr + 1) * N_SSQ_COLS]
                )
            norm = small_pool.tile([128, N_SSQ_COLS], F32, tag="norm")
            nc.scalar.sqrt(norm[:], tsq[:])
            nc.scalar.activation(norm[:], norm[:], AF.Copy, bias=EPS)
            scale = small_pool.tile([128, N_SSQ_COLS], F32, tag="scale")
            nc.vector.reciprocal(scale[:], norm[:])

            # Alternate DVE/ACT so both engines drain the scaling in parallel.
            for gi in range(len(BIG_GROUPS)):
                for r, (r0, rw) in enumerate(ROW_TILES):
                    col = gi * len(ROW_TILES) + r
                    so = souts[(gi, r)]
                    if col % 2 == 0:
                        nc.vector.tensor_scalar_mul(
                            so[:rw, :], so[:rw, :], scale[:rw, col : col + 1]
                        )
                    else:
                        nc.scalar.activation(
                            so[:rw, :], so[:rw, :], AF.Copy,
                            scale=scale[:rw, col : col + 1],
                        )
                    nc.scalar.dma_start(out=om_d[gi][r0 : r0 + rw, :], in_=so[:rw, :])

    nc.compile()
    return nc


_NC = None


def _get_nc():
    global _NC
    if _NC is None:
        _NC = build_program()
    return _NC


def _prep_inputs(lora_tokens, weights):
    """Host-side sharding: gather token groups, transpose contraction onto
    partitions, slice weights per core."""
    lora = np.ascontiguousarray(lora_tokens)

    def pack_supertiles(arr_t):
        # [K, C] -> [K/(128*kb), 128, kb*C]: dense per-partition runs so each
        # super-tile DMA is one fully-contiguous block.
        K, C = arr_t.shape
        kb = KB_BIG
        nsup = K // (128 * kb)
        return np.ascontiguousarray(
            arr_t.reshape(nsup, kb, 128, C).transpose(0, 2, 1, 3).reshape(nsup, 128, kb * C)
        )

    def pack_kv(arr_t):
        # [640, C] -> [128, 5*C]
        K, C = arr_t.shape
        return np.ascontiguousarray(
            arr_t.reshape(5, 128, C).transpose(1, 0, 2).reshape(128, 5 * C)
        )

    shared = {}
    for gi, (off, wname) in enumerate(BIG_GROUPS):
        pos = _positions(off)
        x = lora[:, pos, :].reshape(ROWS, BIG_IND)
        shared[f"xt_{gi}"] = pack_supertiles(x.T.astype(GEMM_NP))
    kv_x = {}
    for gi, (off, wname) in enumerate(KV_GROUPS):
        pos = _positions(off)
        kv_x[gi] = lora[:, pos, :KV_IND].reshape(ROWS, KV_IND)
        shared[f"kvw_{gi}"] = pack_kv(weights[wname].T.astype(GEMM_NP))

    id_pos = np.sort(np.concatenate([_positions(o) for o in IDENTITY_OFFSETS]))
    in_maps = []
    bpc = B // N_CORES
    for c in range(N_CORES):
        m = dict(shared)
        for gi, (off, wname) in enumerate(BIG_GROUPS):
            wsl = weights[wname][c * D_SHARD : (c + 1) * D_SHARD, :]  # [320, 10240]
            m[f"wt_{gi}"] = pack_supertiles(wsl.T.astype(GEMM_NP))
        for gi in range(len(KV_GROUPS)):
            m[f"kvx_{gi}"] = pack_kv(
                kv_x[gi][c * ROWS_PC : (c + 1) * ROWS_PC, :].T.astype(GEMM_NP)
            )
        m["id_x"] = np.ascontiguousarray(
            lora[c * bpc : (c + 1) * bpc, :, :][:, id_pos, :D_MODEL]
        ).reshape(ID_ROWS, D_MODEL)
        in_maps.append(m)
    return in_maps, id_pos


def run(inputs, trace=False):
    nc = _get_nc()
    weights = {k: inputs[k] for k in ("Wk", "Wv", "Wgate", "Wup", "Wdown")}
    in_maps, id_pos = _prep_inputs(inputs["lora_tokens"], weights)
    res = run_bass_kernel_spmd(nc, in_maps, CORE_IDS, trace=trace)

    out = np.zeros((B, NUM_LAYERS * TOKENS_PER_LAYER, D_MODEL), dtype=np.float32)
    bpc = B // N_CORES
    for c in range(N_CORES):
        r = res.results[c]
        out[c * bpc : (c + 1) * bpc, id_pos, :] = r["out_id"].reshape(
            bpc, len(id_pos), D_MODEL
        )
        for gi, (off, wname) in enumerate(BIG_GROUPS):
            pos = _positions(off)
            out[:, pos, c * D_SHARD : (c + 1) * D_SHARD] = r[f"om_{gi}"].reshape(
                B, NUM_LAYERS, D_SHARD
            )
        for gi, (off, wname) in enumerate(KV_GROUPS):
            pos = _positions(off)
            out[c * bpc : (c + 1) * bpc, pos, :] = r[f"kvo_{gi}"].reshape(
                bpc, NUM_LAYERS, D_MODEL
            )
    return out, res


def kernel(**inputs) -> np.ndarray:
    out, _ = run(inputs, trace=False)
    return out



# revision 3
# speedup vs baseline: 1.1203x; 1.1203x over previous
"""Trainium2 Bass kernel for nn_DirectInjectionEncoder (moe_routing).

Strategy (8 NeuronCores):
  - Every projection GEMM (Wgate/Wup/Wdown 10240->2560 and Wk/Wv 640->2560)
    is sharded over the output dim d_model=2560 -> 320 columns per core, so
    each core streams only 1/8 of every weight from HBM.
  - Big-group GEMMs are mixed-precision inside one PSUM accumulation group:
    a fraction F8_FRAC of the contraction runs as e4m3 DoubleRow matmuls
    (2x PE rate, both operands fp8), the rest as e3m4-stationary x
    bf16-moving matmuls (1x PE rate, x still 1 byte/elem). Weights are
    pre-scaled by 50 on the host; the row normalization divides the scale
    back out. Row norms come from fp32 PSUM square-accumulation.
  - Row L2-norms need the full 2560-dim row: each core computes partial
    sums of squares; two tiny AllGathers distribute them. AG1 covers the
    first four shards (up, k, gate, v) and overlaps with the last big
    group's matmuls; AG2 covers only the last group (down), so only its
    ~15us marginal latency plus one 5-column scale pass sits in the tail.
  - Identity tokens (9 of 14 slots/layer, first 2560 dims, no weights) are
    data-parallel over the batch in bf16 both ways; their loads/stores and
    ACT/DVE norm work fill PE-idle and AllGather windows.
  - All outputs are stored as bf16 (the 2e-2 rel-err budget dwarfs bf16
    rounding); the host casts back to fp32 when assembling.
"""

import os
import sys

sys.path.insert(0, "/opt/trn_rl_repo")

import numpy as np
import ml_dtypes

from concourse import bacc, bass, mybir
from concourse.bass_utils import run_bass_kernel_spmd
from concourse.tile import TileContext

D_MODEL = 2560
NUM_LAYERS = 36
TOKENS_PER_LAYER = 14
EPS = 1e-8
B = 16
N_CORES = 8
CORE_IDS = list(range(N_CORES))
D_SHARD = D_MODEL // N_CORES  # 320
ROWS = B * NUM_LAYERS  # 576
ROW_TILES = [(0, 128), (128, 128), (256, 128), (384, 128), (512, 64)]
NRT = len(ROW_TILES)

IDENTITY_OFFSETS = np.array([0, 1, 2, 4, 6, 7, 8, 10, 13])
# interleave kv between big groups; 'down' last so AG2 covers only it
GROUP_ORDER = [
    ("big", 11, "Wup"),
    ("kv", 3, "Wk"),
    ("big", 9, "Wgate"),
    ("kv", 5, "Wv"),
    ("big", 12, "Wdown"),
]
KV_IND = 640
BIG_IND = 10240
N_SSQ = 25  # 5 groups x 5 row tiles, in GROUP_ORDER order
AG1_COLS = 20

ID_ROWS_RAW = (B // N_CORES) * NUM_LAYERS * len(IDENTITY_OFFSETS)  # 648
ID_BLOCKS = 6  # padded to 768 rows = 6 x 128
ID_SUP = 2  # two supertiles of 3 blocks each

W_SCALE = 50.0  # folded into the normalization

F32 = mybir.dt.float32
BF16 = mybir.dt.bfloat16
F8E4 = mybir.dt.float8e4
F8E3 = mybir.dt.float8e3
DR = mybir.MatmulPerfMode.DoubleRow
AF = mybir.ActivationFunctionType

NP_BF16 = ml_dtypes.bfloat16
NP_E4 = ml_dtypes.float8_e4m3
NP_E3 = ml_dtypes.float8_e3m4

# fraction of the big-group contraction dim run as e4m3 DoubleRow
F8_FRAC = float(os.environ.get("KERNEL_F8_FRAC", "0.5"))
N_KT = BIG_IND // 128  # 80 contraction tiles of 128
N_DR = int(round(F8_FRAC * N_KT / 2))  # 256-wide DoubleRow k-tiles
N_E3 = N_KT - 2 * N_DR  # 128-wide e3m4xbf16 k-tiles
K_DR = 256 * N_DR


def _pick_kb(n, cap=8):
    for kb in range(min(n, cap), 0, -1):
        if n % kb == 0:
            return kb
    return 1


KB_DR = _pick_kb(N_DR, 5) if N_DR else 1
KB_E3 = _pick_kb(N_E3, 8) if N_E3 else 1
NSUP_DR = N_DR // KB_DR if N_DR else 0
NSUP_E3 = N_E3 // KB_E3 if N_E3 else 0


def _positions(offset):
    return np.arange(NUM_LAYERS) * TOKENS_PER_LAYER + offset


def build_program():
    nc = bacc.Bacc("TRN2", num_devices=N_CORES)

    big_idx = [i for i, g in enumerate(GROUP_ORDER) if g[0] == "big"]
    kv_idx = [i for i, g in enumerate(GROUP_ORDER) if g[0] == "kv"]

    xdr_d, wdr_d, xe3_d, we3_d, om_d = {}, {}, {}, {}, {}
    for gi in big_idx:
        if NSUP_DR:
            xdr_d[gi] = nc.declare_dram_parameter(f"xdr_{gi}", [NSUP_DR, 128, KB_DR, 2, ROWS], F8E4, isOutput=False)
            wdr_d[gi] = nc.declare_dram_parameter(f"wdr_{gi}", [NSUP_DR, 128, KB_DR, 2, D_SHARD], F8E4, isOutput=False)
        if NSUP_E3:
            xe3_d[gi] = nc.declare_dram_parameter(f"xe3_{gi}", [NSUP_E3, 128, KB_E3, ROWS], F8E3, isOutput=False)
            we3_d[gi] = nc.declare_dram_parameter(f"we3_{gi}", [NSUP_E3, 128, KB_E3, D_SHARD], BF16, isOutput=False)
        om_d[gi] = nc.declare_dram_parameter(f"om_{gi}", [128, NRT * D_SHARD], BF16, isOutput=True)
    kvx_d, kvw_d, kvo_d = {}, {}, {}
    for gi in kv_idx:
        kvx_d[gi] = nc.declare_dram_parameter(f"kvx_{gi}", [128, 5, ROWS], BF16, isOutput=False)
        kvw_d[gi] = nc.declare_dram_parameter(f"kvw_{gi}", [128, 5, D_SHARD], BF16, isOutput=False)
        kvo_d[gi] = nc.declare_dram_parameter(f"kvo_{gi}", [128, NRT * D_SHARD], BF16, isOutput=True)
    idx_d = nc.declare_dram_parameter("id_x", [ID_SUP, 128, 3 * D_MODEL], BF16, isOutput=False)
    ido_d = nc.declare_dram_parameter("out_id", [ID_SUP, 128, 3 * D_MODEL], BF16, isOutput=True)

    with TileContext(nc) as tc:
        with (
            tc.tile_pool(name="xt", bufs=4) as xt_pool,
            tc.tile_pool(name="wt", bufs=4) as wt_pool,
            tc.tile_pool(name="sout", bufs=N_SSQ) as sout_pool,
            tc.tile_pool(name="scr", bufs=2) as scr_pool,
            tc.tile_pool(name="kvp", bufs=4) as kv_pool,
            tc.tile_pool(name="idp", bufs=2) as id_pool,
            tc.tile_pool(name="obf", bufs=6) as obf_pool,
            tc.tile_pool(name="small", bufs=1) as small_pool,
            tc.tile_pool(name="ps", bufs=8, space="PSUM") as psum_pool,
            tc.tile_pool(name="dram", bufs=1, space="DRAM") as dram_pool,
        ):
            ssq = small_pool.tile([128, N_SSQ], F32, tag="ssq")
            nc.vector.memset(ssq[:], 0.0)

            # Warmup AllGather: first collective in a NEFF pays ~60us setup;
            # hide it under the GEMM phase.
            warm_sb = small_pool.tile([1, 16], F32, tag="warmsb")
            nc.vector.memset(warm_sb[:], 0.0)
            warm_in = dram_pool.tile([16], F32, tag="warmci")
            warm_out = dram_pool.tile([N_CORES, 16], F32, tag="warmco")
            nc.gpsimd.dma_start(out=warm_in[:], in_=warm_sb[0, :])
            nc.gpsimd.collective_compute(
                "AllGather",
                mybir.AluOpType.bypass,
                ins=[warm_in.opt()],
                outs=[warm_out.opt()],
                replica_groups=[CORE_IDS],
            )
            nc.gpsimd.dma_start(out=warm_sb[0, :], in_=warm_out[0, :])

            # identity inputs: load both supertiles up front on the ACT ring
            id_tiles = []
            for s in range(ID_SUP):
                it = id_pool.tile([128, 3, D_MODEL], BF16, tag="idin", name=f"idin_{s}")
                nc.scalar.dma_start(out=it[:], in_=idx_d[s].rearrange("p (j c) -> p j c", j=3))
                id_tiles.append(it)
            id_out = []
            for s in range(ID_SUP):
                id_out.append(id_pool.tile([128, 3, D_MODEL], BF16, tag="idout", name=f"idout_{s}"))
            id_ssq = small_pool.tile([128, ID_SUP * 3], F32, tag="idssq")
            id_scale = small_pool.tile([128, ID_SUP * 3], F32, tag="idscale")

            def id_square(s, j):
                iscr = scr_pool.tile([128, D_MODEL], BF16, tag="idscr", name=f"idscr_{s}_{j}")
                nc.scalar.activation(
                    iscr[:], id_tiles[s][:, j, :], AF.Square,
                    accum_out=id_ssq[:, s * 3 + j : s * 3 + j + 1],
                )

            def id_finish(s):
                # norms for the 3 blocks of supertile s, then scale+store
                c0 = s * 3
                nrm = small_pool.tile([128, 3], F32, tag=f"idn{s}", name=f"idnrm_{s}")
                nc.scalar.sqrt(nrm[:], id_ssq[:, c0 : c0 + 3])
                nc.scalar.activation(nrm[:], nrm[:], AF.Copy, bias=EPS)
                nc.vector.reciprocal(id_scale[:, c0 : c0 + 3], nrm[:])
                for j in range(3):
                    nc.vector.tensor_scalar_mul(
                        id_out[s][:, j, :], id_tiles[s][:, j, :],
                        id_scale[:, c0 + j : c0 + j + 1],
                    )
                nc.scalar.dma_start(
                    out=ido_d[s].rearrange("p (j c) -> p j c", j=3), in_=id_out[s][:]
                )

            # kv operand loads (ACT ring), early
            kv_tiles = {}
            for gi in kv_idx:
                kvx = kv_pool.tile([128, 5, ROWS], BF16, tag="kvx", name=f"kvx_{gi}")
                kvw = kv_pool.tile([128, 5, D_SHARD], BF16, tag="kvw", name=f"kvw_{gi}")
                nc.scalar.dma_start(out=kvx[:], in_=kvx_d[gi][:, :, :])
                nc.scalar.dma_start(out=kvw[:], in_=kvw_d[gi][:, :, :])
                kv_tiles[gi] = (kvx, kvw)

            souts = {}

            def big_group(gi):
                ps = [
                    psum_pool.tile([128, D_SHARD], F32, tag="ps", name=f"ps_{gi}_{r}")
                    for r in range(NRT)
                ]
                kt = 0
                for js in range(NSUP_DR):
                    xt = xt_pool.tile([128, KB_DR, 2, ROWS], F8E4, tag="xt", name=f"xdr_{gi}_{js}")
                    wt = wt_pool.tile([128, KB_DR, 2, D_SHARD], F8E4, tag="wt", name=f"wdr_{gi}_{js}")
                    nc.sync.dma_start(out=xt[:], in_=xdr_d[gi][js])
                    nc.gpsimd.dma_start(out=wt[:], in_=wdr_d[gi][js])
                    for j in range(KB_DR):
                        for r, (r0, rw) in enumerate(ROW_TILES):
                            nc.tensor.matmul(
                                ps[r][:rw, :],
                                xt[:, j, :, r0 : r0 + rw],
                                wt[:, j, :, :],
                                start=(kt == 0),
                                stop=False,
                                perf_mode=DR,
                                skip_group_check=True,
                            )
                        kt += 1
                ke = 0
                for js in range(NSUP_E3):
                    xt = xt_pool.tile([128, KB_E3, ROWS], F8E3, tag="xt", name=f"xe3_{gi}_{js}")
                    wt = wt_pool.tile([128, KB_E3, D_SHARD], BF16, tag="wt", name=f"we3_{gi}_{js}")
                    nc.sync.dma_start(out=xt[:], in_=xe3_d[gi][js])
                    nc.gpsimd.dma_start(out=wt[:], in_=we3_d[gi][js])
                    for j in range(KB_E3):
                        last = ke == N_E3 - 1
                        for r, (r0, rw) in enumerate(ROW_TILES):
                            nc.tensor.matmul(
                                ps[r][:rw, :],
                                xt[:, j, r0 : r0 + rw],
                                wt[:, j, :],
                                start=(N_DR == 0 and ke == 0),
                                stop=last,
                                skip_group_check=True,
                            )
                        ke += 1
                return ps

            def kv_group(gi):
                kvx, kvw = kv_tiles[gi]
                ps = [
                    psum_pool.tile([128, D_SHARD], F32, tag="ps", name=f"pkv_{gi}_{r}")
                    for r in range(NRT)
                ]
                for k in range(5):
                    for r, (r0, rw) in enumerate(ROW_TILES):
                        nc.tensor.matmul(
                            ps[r][:rw, :],
                            kvx[:, k, r0 : r0 + rw],
                            kvw[:, k, :],
                            start=(k == 0),
                            stop=(k == 4),
                        )
                return ps

            def drain(slot, gi, ps, keep_psum=False):
                # copy psum->bf16 sbuf (unless final group) + square-accum ssq
                for r, (r0, rw) in enumerate(ROW_TILES):
                    col = slot * NRT + r
                    scr = scr_pool.tile([128, D_SHARD], BF16, tag="scr", name=f"scr_{slot}_{r}")
                    nc.scalar.activation(
                        scr[:rw, :], ps[r][:rw, :], AF.Square,
                        accum_out=ssq[:rw, col : col + 1],
                    )
                    if not keep_psum:
                        so = sout_pool.tile([128, D_SHARD], BF16, tag="sout", name=f"so_{slot}_{r}")
                        nc.vector.tensor_copy(so[:rw, :], ps[r][:rw, :])
                        souts[(slot, r)] = so
                    else:
                        souts[(slot, r)] = ps[r]

            # ---- main pipeline ----
            phase_ps = {}
            for slot, (kind, off, wname) in enumerate(GROUP_ORDER):
                gi = slot
                if kind == "big":
                    ps = big_group(gi)
                else:
                    ps = kv_group(gi)
                drain(slot, gi, ps, keep_psum=(slot == len(GROUP_ORDER) - 1))
                phase_ps[slot] = ps
                if slot == 0:
                    id_square(0, 0)
                    id_square(0, 1)
                elif slot == 1:
                    id_square(0, 2)
                elif slot == 2:
                    id_square(1, 0)
                    id_square(1, 1)
                elif slot == 3:
                    # AG1: first 20 ssq cols (up, k, gate, v)
                    cc_in1 = dram_pool.tile([128, AG1_COLS], F32, tag="ccin1")
                    cc_out1 = dram_pool.tile([N_CORES, 128, AG1_COLS], F32, tag="ccout1")
                    nc.gpsimd.dma_start(out=cc_in1[:], in_=ssq[:, :AG1_COLS])
                    nc.gpsimd.collective_compute(
                        "AllGather",
                        mybir.AluOpType.bypass,
                        ins=[cc_in1.opt()],
                        outs=[cc_out1.opt()],
                        replica_groups=[CORE_IDS],
                    )
                    phase_ps["cc1"] = cc_out1
                    id_square(1, 2)
                    id_finish(0)

            # ---- AG2 for the last group's 5 cols ----
            cc_in2 = dram_pool.tile([128, N_SSQ - AG1_COLS], F32, tag="ccin2")
            cc_out2 = dram_pool.tile([N_CORES, 128, N_SSQ - AG1_COLS], F32, tag="ccout2")
            nc.gpsimd.dma_start(out=cc_in2[:], in_=ssq[:, AG1_COLS:])
            nc.gpsimd.collective_compute(
                "AllGather",
                mybir.AluOpType.bypass,
                ins=[cc_in2.opt()],
                outs=[cc_out2.opt()],
                replica_groups=[CORE_IDS],
            )

            id_finish(1)

            def reduce_scale(cc_out, ncols, col0):
                ag = small_pool.tile([128, N_CORES, ncols], F32, tag=f"ag{col0}", name=f"ag_{col0}")
                nc.scalar.dma_start(out=ag[:], in_=cc_out.rearrange("r p c -> p r c"))
                tsq = small_pool.tile([128, ncols], F32, tag=f"tsq{col0}", name=f"tsq_{col0}")
                nc.vector.tensor_add(tsq[:], ag[:, 0, :], ag[:, 1, :])
                for rr in range(2, N_CORES):
                    nc.vector.tensor_add(tsq[:], tsq[:], ag[:, rr, :])
                nrm = small_pool.tile([128, ncols], F32, tag=f"nrm{col0}", name=f"nrm_{col0}")
                nc.scalar.sqrt(nrm[:], tsq[:])
                nc.scalar.activation(nrm[:], nrm[:], AF.Copy, bias=EPS)
                sc = small_pool.tile([128, ncols], F32, tag=f"sc{col0}", name=f"sc_{col0}")
                nc.vector.reciprocal(sc[:], nrm[:])
                return sc

            def scale_store(slot, sc, sc_col0):
                kind = GROUP_ORDER[slot][0]
                ob = obf_pool.tile([128, NRT, D_SHARD], BF16, tag="obf", name=f"obf_{slot}")
                for r, (r0, rw) in enumerate(ROW_TILES):
                    col = slot * NRT + r - sc_col0
                    src = souts[(slot, r)]
                    if r % 2 == 0:
                        nc.vector.tensor_scalar_mul(
                            ob[:rw, r, :], src[:rw, :], sc[:rw, col : col + 1]
                        )
                    else:
                        nc.scalar.activation(
                            ob[:rw, r, :], src[:rw, :], AF.Copy,
                            scale=sc[:rw, col : col + 1],
                        )
                dst = om_d[slot] if kind == "big" else kvo_d[slot]
                nc.scalar.dma_start(
                    out=dst.rearrange("p (r c) -> p r c", r=NRT), in_=ob[:]
                )

            sc1 = reduce_scale(phase_ps["cc1"], AG1_COLS, 0)
            for slot in range(4):
                scale_store(slot, sc1, 0)

            sc2 = reduce_scale(cc_out2, N_SSQ - AG1_COLS, AG1_COLS)
            scale_store(4, sc2, AG1_COLS)

    nc.compile()
    return nc


_NC = None


def _get_nc():
    global _NC
    if _NC is None:
        _NC = build_program()
    return _NC


def _pack_sup_e3(xT):
    # [K, C] -> [nsup, 128, kb, C]; k = ((js*kb + j)*128 + p)
    K, C = xT.shape
    nsup = K // (128 * KB_E3)
    return np.ascontiguousarray(xT.reshape(nsup, KB_E3, 128, C).transpose(0, 2, 1, 3))


def _pack_sup_dr(xT):
    # [K, C] -> [nsup, 128, kb, 2, C]; k = (((js*kb + j)*2 + i)*128 + p)
    K, C = xT.shape
    nsup = K // (256 * KB_DR)
    return np.ascontiguousarray(xT.reshape(nsup, KB_DR, 2, 128, C).transpose(0, 3, 1, 2, 4))


def _pack_kv(xT):
    # [640, C] -> [128, 5, C]
    K, C = xT.shape
    return np.ascontiguousarray(xT.reshape(5, 128, C).transpose(1, 0, 2))


def _prep_inputs(lora_tokens, weights):
    lora = np.ascontiguousarray(lora_tokens)
    big_idx = [(i, g[1], g[2]) for i, g in enumerate(GROUP_ORDER) if g[0] == "big"]
    kv_idx = [(i, g[1], g[2]) for i, g in enumerate(GROUP_ORDER) if g[0] == "kv"]

    shared = {}
    for gi, off, wname in big_idx:
        pos = _positions(off)
        x = lora[:, pos, :].reshape(ROWS, BIG_IND).T  # [10240, 576]
        if N_DR:
            shared[f"xdr_{gi}"] = _pack_sup_dr(np.clip(x[:K_DR], -240, 240).astype(NP_E4))
        if N_E3:
            shared[f"xe3_{gi}"] = _pack_sup_e3(np.clip(x[K_DR:] * 2.0, -15.0, 15.0).astype(NP_E3))
    kv_xt = {}
    for gi, off, wname in kv_idx:
        pos = _positions(off)
        kv_xt[gi] = lora[:, pos, :KV_IND].reshape(ROWS, KV_IND).T  # [640, 576]
        shared[f"kvx_{gi}"] = _pack_kv(kv_xt[gi].astype(NP_BF16))

    id_pos = np.sort(np.concatenate([_positions(o) for o in IDENTITY_OFFSETS]))
    bpc = B // N_CORES
    in_maps = []
    for c in range(N_CORES):
        m = dict(shared)
        csl = slice(c * D_SHARD, (c + 1) * D_SHARD)
        for gi, off, wname in big_idx:
            wT = weights[wname][csl, :].T  # [10240, 320]
            if N_DR:
                m[f"wdr_{gi}"] = _pack_sup_dr(
                    np.clip(wT[:K_DR] * W_SCALE, -240, 240).astype(NP_E4)
                )
            if N_E3:
                m[f"we3_{gi}"] = _pack_sup_e3((wT[K_DR:] * (W_SCALE / 2.0)).astype(NP_BF16))
        for gi, off, wname in kv_idx:
            m[f"kvw_{gi}"] = _pack_kv(weights[wname][csl, :].T.astype(NP_BF16))
        idx = lora[c * bpc : (c + 1) * bpc, :, :][:, id_pos, :D_MODEL].reshape(
            ID_ROWS_RAW, D_MODEL
        )
        idp = np.ones((ID_SUP * 3 * 128, D_MODEL), dtype=np.float32)
        idp[:ID_ROWS_RAW] = idx
        # row = (s*3 + j)*128 + p  ->  [ID_SUP, 128, 3*D_MODEL]
        m["id_x"] = np.ascontiguousarray(
            idp.reshape(ID_SUP, 3, 128, D_MODEL).transpose(0, 2, 1, 3).reshape(
                ID_SUP, 128, 3 * D_MODEL
            ).astype(NP_BF16)
        )
        in_maps.append(m)
    return in_maps, id_pos


def _unpack_rows(arr128, nrt=NRT, width=D_SHARD):
    # [128, nrt*width] (bf16) -> [nrt*128, width] fp32, caller trims rows
    a = np.asarray(arr128).astype(np.float32).reshape(128, nrt, width)
    return a.transpose(1, 0, 2).reshape(nrt * 128, width)


def run(inputs, trace=False):
    nc = _get_nc()
    weights = {k: inputs[k] for k in ("Wk", "Wv", "Wgate", "Wup", "Wdown")}
    in_maps, id_pos = _prep_inputs(inputs["lora_tokens"], weights)
    res = run_bass_kernel_spmd(nc, in_maps, CORE_IDS, trace=trace)

    out = np.zeros((B, NUM_LAYERS * TOKENS_PER_LAYER, D_MODEL), dtype=np.float32)
    bpc = B // N_CORES
    for c in range(N_CORES):
        r = res.results[c]
        csl = slice(c * D_SHARD, (c + 1) * D_SHARD)
        for slot, (kind, off, wname) in enumerate(GROUP_ORDER):
            pos = _positions(off)
            key = f"om_{slot}" if kind == "big" else f"kvo_{slot}"
            rows = _unpack_rows(r[key])[:ROWS]
            out[:, pos, csl] = rows.reshape(B, NUM_LAYERS, D_SHARD)
        ido = np.asarray(r["out_id"]).astype(np.float32).reshape(ID_SUP, 128, 3, D_MODEL)
        ido = ido.transpose(0, 2, 1, 3).reshape(ID_SUP * 3 * 128, D_MODEL)[:ID_ROWS_RAW]
        out[c * bpc : (c + 1) * bpc, id_pos, :] = ido.reshape(bpc, len(id_pos), D_MODEL)
    return out, res


def kernel(**inputs) -> np.ndarray:
    out, _ = run(inputs, trace=False)
    return out


# revision 4
# speedup vs baseline: 1.7039x; 1.5209x over previous
"""Trainium2 Bass kernel for nn_DirectInjectionEncoder (moe_routing).

Strategy (8 NeuronCores):
  - Every projection GEMM (Wgate/Wup/Wdown 10240->2560 and Wk/Wv 640->2560)
    is sharded over the output dim d_model=2560 -> 320 columns per core, so
    each core streams only 1/8 of every weight from HBM.
  - Big-group GEMMs are mixed-precision inside one PSUM accumulation group:
    a fraction F8_FRAC of the contraction runs as e4m3 DoubleRow matmuls
    (2x PE rate, both operands fp8), the rest as e3m4-stationary x
    bf16-moving matmuls (1x PE rate, x still 1 byte/elem). Weights are
    pre-scaled by 50 on the host; the row normalization divides the scale
    back out. Row norms come from fp32 PSUM square-accumulation.
  - Row L2-norms need the full 2560-dim row: each core computes partial
    sums of squares; two tiny AllGathers distribute them. AG1 covers the
    first four shards (up, k, gate, v) and overlaps with the last big
    group's matmuls; AG2 covers only the last group (down), so only its
    ~15us marginal latency plus one 5-column scale pass sits in the tail.
  - Identity tokens (9 of 14 slots/layer, first 2560 dims, no weights) are
    data-parallel over the batch in bf16 both ways; their loads/stores and
    ACT/DVE norm work fill PE-idle and AllGather windows.
  - All outputs are stored as bf16 (the 2e-2 rel-err budget dwarfs bf16
    rounding); the host casts back to fp32 when assembling.
"""

import os
import sys

sys.path.insert(0, "/opt/trn_rl_repo")

import numpy as np
import ml_dtypes

from concourse import bacc, bass, mybir
from concourse.bass_utils import run_bass_kernel_spmd
from concourse.tile import TileContext

D_MODEL = 2560
NUM_LAYERS = 36
TOKENS_PER_LAYER = 14
EPS = 1e-8
B = 16
N_CORES = 8
CORE_IDS = list(range(N_CORES))
D_SHARD = D_MODEL // N_CORES  # 320
ROWS = B * NUM_LAYERS  # 576
ROW_TILES = [(0, 128), (128, 128), (256, 128), (384, 128), (512, 64)]
NRT = len(ROW_TILES)

IDENTITY_OFFSETS = np.array([0, 1, 2, 4, 6, 7, 8, 10, 13])
# interleave kv between big groups; 'down' last so AG2 covers only it
GROUP_ORDER = [
    ("big", 11, "Wup"),
    ("kv", 3, "Wk"),
    ("big", 9, "Wgate"),
    ("kv", 5, "Wv"),
    ("big", 12, "Wdown"),
]
KV_IND = 640
BIG_IND = 10240
N_SSQ = 25  # 5 groups x 5 row tiles, in GROUP_ORDER order
AG1_COLS = 20

ID_ROWS_RAW = (B // N_CORES) * NUM_LAYERS * len(IDENTITY_OFFSETS)  # 648
ID_BLOCKS = 6  # padded to 768 rows = 6 x 128
ID_SUP = 2  # two supertiles of 3 blocks each

W_SCALE = 50.0  # folded into the normalization

F32 = mybir.dt.float32
BF16 = mybir.dt.bfloat16
F8E4 = mybir.dt.float8e4
F8E3 = mybir.dt.float8e3
DR = mybir.MatmulPerfMode.DoubleRow
AF = mybir.ActivationFunctionType

NP_BF16 = ml_dtypes.bfloat16
NP_E4 = ml_dtypes.float8_e4m3
NP_E3 = ml_dtypes.float8_e3m4

# fraction of the big-group contraction dim run as e4m3 DoubleRow
F8_FRAC = float(os.environ.get("KERNEL_F8_FRAC", "1.0"))
N_KT = BIG_IND // 128  # 80 contraction tiles of 128
N_DR = int(round(F8_FRAC * N_KT / 2))  # 256-wide DoubleRow k-tiles
N_E3 = N_KT - 2 * N_DR  # 128-wide e3m4xbf16 k-tiles
K_DR = 256 * N_DR


def _pick_kb(n, cap=8):
    for kb in range(min(n, cap), 0, -1):
        if n % kb == 0:
            return kb
    return 1


KB_DR = _pick_kb(N_DR, 5) if N_DR else 1
KB_E3 = _pick_kb(N_E3, 8) if N_E3 else 1
NSUP_DR = N_DR // KB_DR if N_DR else 0
NSUP_E3 = N_E3 // KB_E3 if N_E3 else 0


def _positions(offset):
    return np.arange(NUM_LAYERS) * TOKENS_PER_LAYER + offset


def build_program():
    nc = bacc.Bacc("TRN2", num_devices=N_CORES)

    big_idx = [i for i, g in enumerate(GROUP_ORDER) if g[0] == "big"]
    kv_idx = [i for i, g in enumerate(GROUP_ORDER) if g[0] == "kv"]

    xdr_d, wdr_d, xe3_d, we3_d, om_d = {}, {}, {}, {}, {}
    for gi in big_idx:
        if NSUP_DR:
            xdr_d[gi] = nc.declare_dram_parameter(f"xdr_{gi}", [NSUP_DR, 128, KB_DR, 2, ROWS], F8E4, isOutput=False)
            wdr_d[gi] = nc.declare_dram_parameter(f"wdr_{gi}", [NSUP_DR, 128, KB_DR, 2, D_SHARD], F8E4, isOutput=False)
        if NSUP_E3:
            xe3_d[gi] = nc.declare_dram_parameter(f"xe3_{gi}", [NSUP_E3, 128, KB_E3, ROWS], F8E3, isOutput=False)
            we3_d[gi] = nc.declare_dram_parameter(f"we3_{gi}", [NSUP_E3, 128, KB_E3, D_SHARD], BF16, isOutput=False)
        om_d[gi] = nc.declare_dram_parameter(f"om_{gi}", [128, NRT * D_SHARD], BF16, isOutput=True)
    kvx_d, kvw_d, kvo_d = {}, {}, {}
    for gi in kv_idx:
        kvx_d[gi] = nc.declare_dram_parameter(f"kvx_{gi}", [128, 5, ROWS], BF16, isOutput=False)
        kvw_d[gi] = nc.declare_dram_parameter(f"kvw_{gi}", [128, 5, D_SHARD], BF16, isOutput=False)
        kvo_d[gi] = nc.declare_dram_parameter(f"kvo_{gi}", [128, NRT * D_SHARD], BF16, isOutput=True)
    idx_d = nc.declare_dram_parameter("id_x", [ID_SUP, 128, 3 * D_MODEL], BF16, isOutput=False)
    ido_d = nc.declare_dram_parameter("out_id", [ID_SUP, 128, 3 * D_MODEL], BF16, isOutput=True)

    with TileContext(nc) as tc:
        with (
            tc.tile_pool(name="xt", bufs=4) as xt_pool,
            tc.tile_pool(name="wt", bufs=4) as wt_pool,
            tc.tile_pool(name="sout", bufs=N_SSQ) as sout_pool,
            tc.tile_pool(name="scr", bufs=2) as scr_pool,
            tc.tile_pool(name="kvp", bufs=4) as kv_pool,
            tc.tile_pool(name="idp", bufs=2) as id_pool,
            tc.tile_pool(name="obf", bufs=6) as obf_pool,
            tc.tile_pool(name="small", bufs=1) as small_pool,
            tc.tile_pool(name="ps", bufs=8, space="PSUM") as psum_pool,
            tc.tile_pool(name="dram", bufs=1, space="DRAM") as dram_pool,
        ):
            ssq = small_pool.tile([128, N_SSQ], F32, tag="ssq")
            nc.vector.memset(ssq[:], 0.0)

            # Warmup AllGather: first collective in a NEFF pays ~60us setup;
            # hide it under the GEMM phase.
            warm_sb = small_pool.tile([1, 16], F32, tag="warmsb")
            nc.vector.memset(warm_sb[:], 0.0)
            warm_in = dram_pool.tile([16], F32, tag="warmci")
            warm_out = dram_pool.tile([N_CORES, 16], F32, tag="warmco")
            nc.gpsimd.dma_start(out=warm_in[:], in_=warm_sb[0, :])
            nc.gpsimd.collective_compute(
                "AllGather",
                mybir.AluOpType.bypass,
                ins=[warm_in.opt()],
                outs=[warm_out.opt()],
                replica_groups=[CORE_IDS],
            )

            # identity inputs: load both supertiles up front on the ACT ring
            id_tiles = []
            for s in range(ID_SUP):
                it = id_pool.tile([128, 3, D_MODEL], BF16, tag="idin", name=f"idin_{s}")
                nc.scalar.dma_start(out=it[:], in_=idx_d[s].rearrange("p (j c) -> p j c", j=3))
                id_tiles.append(it)
            id_out = []
            for s in range(ID_SUP):
                id_out.append(id_pool.tile([128, 3, D_MODEL], BF16, tag="idout", name=f"idout_{s}"))
            id_ssq = small_pool.tile([128, ID_SUP * 3], F32, tag="idssq")
            id_scale = small_pool.tile([128, ID_SUP * 3], F32, tag="idscale")

            def id_square(s, j):
                iscr = scr_pool.tile([128, D_MODEL], BF16, tag="idscr", name=f"idscr_{s}_{j}")
                nc.scalar.activation(
                    iscr[:], id_tiles[s][:, j, :], AF.Square,
                    accum_out=id_ssq[:, s * 3 + j : s * 3 + j + 1],
                )

            def id_finish(s):
                # norms for the 3 blocks of supertile s, then scale+store
                c0 = s * 3
                nrm = small_pool.tile([128, 3], F32, tag=f"idn{s}", name=f"idnrm_{s}")
                nc.scalar.sqrt(nrm[:], id_ssq[:, c0 : c0 + 3])
                nc.scalar.activation(nrm[:], nrm[:], AF.Copy, bias=EPS)
                nc.vector.reciprocal(id_scale[:, c0 : c0 + 3], nrm[:])
                for j in range(3):
                    nc.vector.tensor_scalar_mul(
                        id_out[s][:, j, :], id_tiles[s][:, j, :],
                        id_scale[:, c0 + j : c0 + j + 1],
                    )
                nc.scalar.dma_start(
                    out=ido_d[s].rearrange("p (j c) -> p j c", j=3), in_=id_out[s][:]
                )

            # kv operand loads (ACT ring), early
            kv_tiles = {}
            for gi in kv_idx:
                kvx = kv_pool.tile([128, 5, ROWS], BF16, tag="kvx", name=f"kvx_{gi}")
                kvw = kv_pool.tile([128, 5, D_SHARD], BF16, tag="kvw", name=f"kvw_{gi}")
                nc.scalar.dma_start(out=kvx[:], in_=kvx_d[gi][:, :, :])
                nc.scalar.dma_start(out=kvw[:], in_=kvw_d[gi][:, :, :])
                kv_tiles[gi] = (kvx, kvw)

            souts = {}

            def big_group(gi):
                ps = [
                    psum_pool.tile([128, D_SHARD], F32, tag="ps", name=f"ps_{gi}_{r}")
                    for r in range(NRT)
                ]
                kt = 0
                for js in range(NSUP_DR):
                    xt = xt_pool.tile([128, KB_DR, 2, ROWS], F8E4, tag="xt", name=f"xdr_{gi}_{js}")
                    wt = wt_pool.tile([128, KB_DR, 2, D_SHARD], F8E4, tag="wt", name=f"wdr_{gi}_{js}")
                    nc.sync.dma_start(out=xt[:], in_=xdr_d[gi][js])
                    nc.sync.dma_start(out=wt[:], in_=wdr_d[gi][js])
                    for j in range(KB_DR):
                        for r, (r0, rw) in enumerate(ROW_TILES):
                            nc.tensor.matmul(
                                ps[r][:rw, :],
                                xt[:, j, :, r0 : r0 + rw],
                                wt[:, j, :, :],
                                start=(kt == 0),
                                stop=(N_E3 == 0 and kt == N_DR - 1),
                                perf_mode=DR,
                                skip_group_check=True,
                            )
                        kt += 1
                ke = 0
                for js in range(NSUP_E3):
                    xt = xt_pool.tile([128, KB_E3, ROWS], F8E3, tag="xt", name=f"xe3_{gi}_{js}")
                    wt = wt_pool.tile([128, KB_E3, D_SHARD], BF16, tag="wt", name=f"we3_{gi}_{js}")
                    nc.sync.dma_start(out=xt[:], in_=xe3_d[gi][js])
                    nc.sync.dma_start(out=wt[:], in_=we3_d[gi][js])
                    for j in range(KB_E3):
                        last = ke == N_E3 - 1
                        for r, (r0, rw) in enumerate(ROW_TILES):
                            nc.tensor.matmul(
                                ps[r][:rw, :],
                                xt[:, j, r0 : r0 + rw],
                                wt[:, j, :],
                                start=(N_DR == 0 and ke == 0),
                                stop=last,
                                skip_group_check=True,
                            )
                        ke += 1
                return ps

            def kv_group(gi):
                kvx, kvw = kv_tiles[gi]
                ps = [
                    psum_pool.tile([128, D_SHARD], F32, tag="ps", name=f"pkv_{gi}_{r}")
                    for r in range(NRT)
                ]
                for k in range(5):
                    for r, (r0, rw) in enumerate(ROW_TILES):
                        nc.tensor.matmul(
                            ps[r][:rw, :],
                            kvx[:, k, r0 : r0 + rw],
                            kvw[:, k, :],
                            start=(k == 0),
                            stop=(k == 4),
                        )
                return ps

            def drain(slot, gi, ps, keep_psum=False):
                # copy psum->bf16 sbuf (unless final group) + square-accum ssq
                for r, (r0, rw) in enumerate(ROW_TILES):
                    col = slot * NRT + r
                    scr = scr_pool.tile([128, D_SHARD], BF16, tag="scr", name=f"scr_{slot}_{r}")
                    nc.scalar.activation(
                        scr[:rw, :], ps[r][:rw, :], AF.Square,
                        accum_out=ssq[:rw, col : col + 1],
                    )
                    if not keep_psum:
                        so = sout_pool.tile([128, D_SHARD], BF16, tag="sout", name=f"so_{slot}_{r}")
                        nc.vector.tensor_copy(so[:rw, :], ps[r][:rw, :])
                        souts[(slot, r)] = so
                    else:
                        souts[(slot, r)] = ps[r]

            # ---- main pipeline ----
            phase_ps = {}
            for slot, (kind, off, wname) in enumerate(GROUP_ORDER):
                gi = slot
                if kind == "big":
                    ps = big_group(gi)
                else:
                    ps = kv_group(gi)
                drain(slot, gi, ps, keep_psum=(slot == len(GROUP_ORDER) - 1))
                phase_ps[slot] = ps
                if slot == 0:
                    id_square(0, 0)
                    id_square(0, 1)
                elif slot == 1:
                    id_square(0, 2)
                elif slot == 2:
                    id_square(1, 0)
                    id_square(1, 1)
                elif slot == 3:
                    # AG1: first 20 ssq cols (up, k, gate, v)
                    cc_in1 = dram_pool.tile([128, AG1_COLS], F32, tag="ccin1")
                    cc_out1 = dram_pool.tile([N_CORES, 128, AG1_COLS], F32, tag="ccout1")
                    nc.gpsimd.dma_start(out=cc_in1[:], in_=ssq[:, :AG1_COLS])
                    nc.gpsimd.collective_compute(
                        "AllGather",
                        mybir.AluOpType.bypass,
                        ins=[cc_in1.opt()],
                        outs=[cc_out1.opt()],
                        replica_groups=[CORE_IDS],
                    )
                    phase_ps["cc1"] = cc_out1
                    id_square(1, 2)
                    id_finish(0)

            # ---- AG2 for the last group's 5 cols ----
            cc_in2 = dram_pool.tile([128, N_SSQ - AG1_COLS], F32, tag="ccin2")
            cc_out2 = dram_pool.tile([N_CORES, 128, N_SSQ - AG1_COLS], F32, tag="ccout2")
            nc.gpsimd.dma_start(out=cc_in2[:], in_=ssq[:, AG1_COLS:])
            nc.gpsimd.collective_compute(
                "AllGather",
                mybir.AluOpType.bypass,
                ins=[cc_in2.opt()],
                outs=[cc_out2.opt()],
                replica_groups=[CORE_IDS],
            )

            id_finish(1)

            def reduce_scale(cc_out, ncols, col0):
                ag = small_pool.tile([128, N_CORES, ncols], F32, tag=f"ag{col0}", name=f"ag_{col0}")
                nc.scalar.dma_start(out=ag[:], in_=cc_out.rearrange("r p c -> p r c"))
                tsq = small_pool.tile([128, ncols], F32, tag=f"tsq{col0}", name=f"tsq_{col0}")
                nc.vector.tensor_add(tsq[:], ag[:, 0, :], ag[:, 1, :])
                for rr in range(2, N_CORES):
                    nc.vector.tensor_add(tsq[:], tsq[:], ag[:, rr, :])
                nrm = small_pool.tile([128, ncols], F32, tag=f"nrm{col0}", name=f"nrm_{col0}")
                nc.scalar.sqrt(nrm[:], tsq[:])
                nc.scalar.activation(nrm[:], nrm[:], AF.Copy, bias=EPS)
                sc = small_pool.tile([128, ncols], F32, tag=f"sc{col0}", name=f"sc_{col0}")
                nc.vector.reciprocal(sc[:], nrm[:])
                return sc

            def scale_store(slot, sc, sc_col0):
                kind = GROUP_ORDER[slot][0]
                ob = obf_pool.tile([128, NRT, D_SHARD], BF16, tag="obf", name=f"obf_{slot}")
                for r, (r0, rw) in enumerate(ROW_TILES):
                    col = slot * NRT + r - sc_col0
                    src = souts[(slot, r)]
                    if r % 2 == 0:
                        nc.vector.tensor_scalar_mul(
                            ob[:rw, r, :], src[:rw, :], sc[:rw, col : col + 1]
                        )
                    else:
                        nc.scalar.activation(
                            ob[:rw, r, :], src[:rw, :], AF.Copy,
                            scale=sc[:rw, col : col + 1],
                        )
                dst = om_d[slot] if kind == "big" else kvo_d[slot]
                nc.scalar.dma_start(
                    out=dst.rearrange("p (r c) -> p r c", r=NRT), in_=ob[:]
                )

            sc1 = reduce_scale(phase_ps["cc1"], AG1_COLS, 0)
            for slot in range(4):
                scale_store(slot, sc1, 0)

            sc2 = reduce_scale(cc_out2, N_SSQ - AG1_COLS, AG1_COLS)
            scale_store(4, sc2, AG1_COLS)

    nc.compile()
    return nc


_NC = None


def _get_nc():
    global _NC
    if _NC is None:
        _NC = build_program()
    return _NC


def _pack_sup_e3(xT):
    # [K, C] -> [nsup, 128, kb, C]; k = ((js*kb + j)*128 + p)
    K, C = xT.shape
    nsup = K // (128 * KB_E3)
    return np.ascontiguousarray(xT.reshape(nsup, KB_E3, 128, C).transpose(0, 2, 1, 3))


def _pack_sup_dr(xT):
    # [K, C] -> [nsup, 128, kb, 2, C]; k = (((js*kb + j)*2 + i)*128 + p)
    K, C = xT.shape
    nsup = K // (256 * KB_DR)
    return np.ascontiguousarray(xT.reshape(nsup, KB_DR, 2, 128, C).transpose(0, 3, 1, 2, 4))


def _pack_kv(xT):
    # [640, C] -> [128, 5, C]
    K, C = xT.shape
    return np.ascontiguousarray(xT.reshape(5, 128, C).transpose(1, 0, 2))


def _prep_inputs(lora_tokens, weights):
    lora = np.ascontiguousarray(lora_tokens)
    big_idx = [(i, g[1], g[2]) for i, g in enumerate(GROUP_ORDER) if g[0] == "big"]
    kv_idx = [(i, g[1], g[2]) for i, g in enumerate(GROUP_ORDER) if g[0] == "kv"]

    shared = {}
    for gi, off, wname in big_idx:
        pos = _positions(off)
        x = lora[:, pos, :].reshape(ROWS, BIG_IND).T  # [10240, 576]
        if N_DR:
            shared[f"xdr_{gi}"] = _pack_sup_dr(np.clip(x[:K_DR], -240, 240).astype(NP_E4))
        if N_E3:
            shared[f"xe3_{gi}"] = _pack_sup_e3(np.clip(x[K_DR:] * 2.0, -15.0, 15.0).astype(NP_E3))
    kv_xt = {}
    for gi, off, wname in kv_idx:
        pos = _positions(off)
        kv_xt[gi] = lora[:, pos, :KV_IND].reshape(ROWS, KV_IND).T  # [640, 576]
        shared[f"kvx_{gi}"] = _pack_kv(kv_xt[gi].astype(NP_BF16))

    id_pos = np.sort(np.concatenate([_positions(o) for o in IDENTITY_OFFSETS]))
    bpc = B // N_CORES
    in_maps = []
    for c in range(N_CORES):
        m = dict(shared)
        csl = slice(c * D_SHARD, (c + 1) * D_SHARD)
        for gi, off, wname in big_idx:
            wT = weights[wname][csl, :].T  # [10240, 320]
            if N_DR:
                m[f"wdr_{gi}"] = _pack_sup_dr(
                    np.clip(wT[:K_DR] * W_SCALE, -240, 240).astype(NP_E4)
                )
            if N_E3:
                m[f"we3_{gi}"] = _pack_sup_e3((wT[K_DR:] * (W_SCALE / 2.0)).astype(NP_BF16))
        for gi, off, wname in kv_idx:
            m[f"kvw_{gi}"] = _pack_kv(weights[wname][csl, :].T.astype(NP_BF16))
        idx = lora[c * bpc : (c + 1) * bpc, :, :][:, id_pos, :D_MODEL].reshape(
            ID_ROWS_RAW, D_MODEL
        )
        idp = np.ones((ID_SUP * 3 * 128, D_MODEL), dtype=np.float32)
        idp[:ID_ROWS_RAW] = idx
        # row = (s*3 + j)*128 + p  ->  [ID_SUP, 128, 3*D_MODEL]
        m["id_x"] = np.ascontiguousarray(
            idp.reshape(ID_SUP, 3, 128, D_MODEL).transpose(0, 2, 1, 3).reshape(
                ID_SUP, 128, 3 * D_MODEL
            ).astype(NP_BF16)
        )
        in_maps.append(m)
    return in_maps, id_pos


def _unpack_rows(arr128, nrt=NRT, width=D_SHARD):
    # [128, nrt*width] (bf16) -> [nrt*128, width] fp32, caller trims rows
    a = np.asarray(arr128).astype(np.float32).reshape(128, nrt, width)
    return a.transpose(1, 0, 2).reshape(nrt * 128, width)


def run(inputs, trace=False):
    nc = _get_nc()
    weights = {k: inputs[k] for k in ("Wk", "Wv", "Wgate", "Wup", "Wdown")}
    in_maps, id_pos = _prep_inputs(inputs["lora_tokens"], weights)
    res = run_bass_kernel_spmd(nc, in_maps, CORE_IDS, trace=trace)

    out = np.zeros((B, NUM_LAYERS * TOKENS_PER_LAYER, D_MODEL), dtype=np.float32)
    bpc = B // N_CORES
    for c in range(N_CORES):
        r = res.results[c]
        csl = slice(c * D_SHARD, (c + 1) * D_SHARD)
        for slot, (kind, off, wname) in enumerate(GROUP_ORDER):
            pos = _positions(off)
            key = f"om_{slot}" if kind == "big" else f"kvo_{slot}"
            rows = _unpack_rows(r[key])[:ROWS]
            out[:, pos, csl] = rows.reshape(B, NUM_LAYERS, D_SHARD)
        ido = np.asarray(r["out_id"]).astype(np.float32).reshape(ID_SUP, 128, 3, D_MODEL)
        ido = ido.transpose(0, 2, 1, 3).reshape(ID_SUP * 3 * 128, D_MODEL)[:ID_ROWS_RAW]
        out[c * bpc : (c + 1) * bpc, id_pos, :] = ido.reshape(bpc, len(id_pos), D_MODEL)
    return out, res


def kernel(**inputs) -> np.ndarray:
    out, _ = run(inputs, trace=False)
    return out
